# revision 24
# baseline (speedup 1.0000x reference)
"""Locally-connected graph-conv kernel for Trainium2 (Bass/Tile).

Computes out[b,t,m] = sum_n x[b,t,n] * (S*W)[n,m] + bias[m] for
x [64, 2048, 208], W/S [208, 208], bias [208].

The ring-graph support S is a +-4 band (mod 208), so each half of the
output nodes only needs a 112-row slice of the contraction dim. With a
rotated node layout (row j holds node (j-4) mod 208, 216 rows total):
  block 0 (m 0..103):   rotated rows   0..111
  block 1 (m 104..207): rotated rows 104..215
Each output block is a SINGLE [112,104] x [112,512] matmul with the
host-premasked weight block stationary in the PE array and x^T streaming
as the moving operand.

Everything that touches HBM is bf16 (PSUM accumulation stays fp32): the
2e-2 rel-err budget dwarfs bf16 rounding (~5e-3) and it halves DMA bytes
vs fp32. HBM per NeuronCore is ~358 GB/s and ramps up over the first
~20 us, so the ~14.9 MB/core of traffic floors the kernel at ~45 us.
The two output blocks form fully decoupled pipelines that share only
the PE and the load ring:
  block 0: Sync-ring load -> matmul -> VectorE evict -> Sync-ring store
  block 1: Sync-ring load -> matmul -> ScalarE evict -> Scalar-ring store
so neither evicting engine ever waits on the other. Block 0 stores are
queued on the Sync ring AFTER all loads (FIFO keeps them out of the
loads' way; the ring is otherwise idle then). Weights and bias are
padded to >=1KB DMA rows (tiny-descriptor DMAs crawl at ~27 GB/s) and
ride the Scalar ring ahead of the stores; GpSimd SWDGE is avoided (its
completion semaphore fires ~7 us late). PSUM->SBUF eviction is stuck at
1 elem/lane/cycle (fp32 PSUM source), which is why the eviction work is
split across both engines. A few dummy matmuls on the weight tile right
after it lands warm the PE HAM clock gate (cold 1.2 GHz -> warm 2.4).
The host transposes y^T back at gather.
"""

import numpy as np
import ml_dtypes
from contextlib import ExitStack

import concourse.bacc as bacc
import concourse.mybir as mybir
import concourse.tile as tile
from concourse.bass_utils import run_bass_kernel_spmd

N = 208                      # nodes
HALF = 104                   # output nodes per block
K = 4                        # band half-width of S
NH = 2 * K + HALF            # 112 contraction rows per block (halo incl.)
NR = N + 2 * K               # 216 rotated rows
WPAD = 1024                  # wh DRAM row padding (2 KB rows -> fast DMA)
BPAD = 256                   # bias DRAM row padding (1 KB f32 rows)
N_CORES = 8
B, T = 64, 2048
ROWS_TOTAL = B * T           # 131072
SHARD = ROWS_TOTAL // N_CORES    # 16384 rows per core
TB = 512                     # moving-block columns per matmul (fp32 PSUM max)
TB2 = 2 * TB                 # eviction group (2 PSUM banks)
CHUNKS = [2048, 2048, 4096, 4096, 2048, 2048]   # t-cols per pipeline chunk
assert sum(CHUNKS) == SHARD
N_DUMMY = 4                  # PE warm-up matmuls on the weight tile

FP32 = mybir.dt.float32
BF16 = mybir.dt.bfloat16
NP_BF16 = ml_dtypes.bfloat16
IDENT = mybir.ActivationFunctionType.Identity

# halo row order (indices into the [208] node dim) for each block
ROWS0 = list(range(N - K, N)) + list(range(0, HALF + K))          # 112
ROWS1 = list(range(HALF - K, N)) + list(range(0, K))              # 112

_CACHE = {}
LAST_RESULTS = None          # BassKernelResults of the most recent run


def _kernel_body(tc):
    nc = tc.nc
    # rotated x: row j = node (j-4) mod 208; block0 = rows 0:112,
    # block1 = rows 104:216
    x_d = nc.dram_tensor("xh", [NR, SHARD], BF16, kind="ExternalInput").ap()
    w_d = nc.dram_tensor("wh", [NH, WPAD], BF16, kind="ExternalInput").ap()
    b_d = nc.dram_tensor("bias", [2 * NH, BPAD], FP32, kind="ExternalInput").ap()
    o_d = nc.dram_tensor("outt", [2 * NH, SHARD], BF16, kind="ExternalOutput").ap()

    with ExitStack() as ctx:
        const = ctx.enter_context(tc.tile_pool(name="const", bufs=1))

        # One-time setup on the Scalar ring (it carries only stores
        # later): weights first (the warm-up matmuls need them), then
        # bias -- delays no x load, lands before the first eviction.
        wh = const.tile([NH, WPAD], BF16, tag="wh")
        nc.scalar.dma_start(wh, w_d)
        bA = const.tile([NH, BPAD], FP32, tag="bA")
        bB = const.tile([NH, BPAD], FP32, tag="bB")
        nc.scalar.dma_start(bA, b_d[0:NH, :])
        nc.scalar.dma_start(bB, b_d[NH : 2 * NH, :])
        bAc = bA[0:HALF, 0:1]
        bBc = bB[0:HALF, 0:1]

        o0p = ctx.enter_context(tc.tile_pool(name="o0p", bufs=3))
        o1p = ctx.enter_context(tc.tile_pool(name="o1p", bufs=3))
        ps0p = ctx.enter_context(tc.tile_pool(name="ps0p", bufs=2, space="PSUM"))
        ps1p = ctx.enter_context(tc.tile_pool(name="ps1p", bufs=2, space="PSUM"))

        # All x loads up-front into persistent tiles, split across both
        # rings by chunk parity (two concurrent load streams pull HBM
        # harder during the early ramp; block order per chunk preserved):
        # even chunks on Sync, odd on Scalar. Bias rides Sync after the
        # first chunk.
        xts = []
        col = 0
        for c, csz in enumerate(CHUNKS):
            lsl = slice(col, col + csz)
            xh0 = const.tile([NH, csz], BF16, tag=f"xh0_{c}")
            xh1 = const.tile([NH, csz], BF16, tag=f"xh1_{c}")
            eng = nc.sync if c % 2 == 0 else nc.scalar
            eng.dma_start(xh0, x_d[0:NH, lsl])
            eng.dma_start(xh1, x_d[HALF:NR, lsl])
            xts.append((xh0, xh1, col, csz))
            col += csz

        # PE warm-up: HAM un-throttles (1.2 -> 2.4 GHz) after ~3.4us of
        # sustained busy; burn idle pre-data time on the weight tile.
        for _ in range(N_DUMMY):
            psd = ps0p.tile([HALF, TB2], FP32, tag="ps0")
            nc.tensor.matmul(psd[:, 0:TB], wh[:, 0:HALF], wh[:, 0:TB], start=True, stop=True)

        for c, (xh0, xh1, col, csz) in enumerate(xts):
            tsl = slice(col, col + csz)
            o0_t = o0p.tile([NH, max(CHUNKS)], BF16, tag="o0")
            o1_t = o1p.tile([NH, max(CHUNKS)], BF16, tag="o1")
            for s in range(csz // TB2):
                g = slice(s * TB2, (s + 1) * TB2)
                ga = slice(s * TB2, s * TB2 + TB)
                gb = slice(s * TB2 + TB, (s + 1) * TB2)
                # [104, 1024] PSUM tiles (2 banks); one matmul per bank
                ps0 = ps0p.tile([HALF, TB2], FP32, tag="ps0")
                nc.tensor.matmul(ps0[:, 0:TB], wh[:, 0:HALF], xh0[:, ga], start=True, stop=True)
                nc.tensor.matmul(ps0[:, TB:TB2], wh[:, 0:HALF], xh0[:, gb], start=True, stop=True)
                ps1 = ps1p.tile([HALF, TB2], FP32, tag="ps1")
                nc.tensor.matmul(ps1[:, 0:TB], wh[:, HALF:N], xh1[:, ga], start=True, stop=True)
                nc.tensor.matmul(ps1[:, TB:TB2], wh[:, HALF:N], xh1[:, gb], start=True, stop=True)
                # evictions split across engines; both fuse bias + fp32->bf16
                nc.vector.tensor_scalar_add(o0_t[0:HALF, g], ps0, bAc)
                nc.scalar.activation(o1_t[0:HALF, g], ps1, IDENT, bias=bBc)
            # decoupled stores: block0 on the Sync ring (queued after all
            # loads), block1 on the Scalar ring
            nc.sync.dma_start(o_d[0:NH, tsl], o0_t[:, 0:csz])
            nc.scalar.dma_start(o_d[NH : 2 * NH, tsl], o1_t[:, 0:csz])


def _build():
    nc = bacc.Bacc(
        "TRN2",
        target_bir_lowering=False,
        debug=False,
        num_devices=N_CORES,
    )
    with tile.TileContext(nc) as tc:
        _kernel_body(tc)
    nc.compile()
    return nc


def kernel(x, W, b, S):
    global LAST_RESULTS
    nc = _CACHE.get("nc")
    if nc is None:
        nc = _build()
        _CACHE["nc"] = nc

    xf = np.asarray(x, np.float32).reshape(ROWS_TOTAL, N)
    SW = (np.asarray(S, np.float32) * np.asarray(W, np.float32))
    wh = np.zeros((NH, WPAD), NP_BF16)
    wh[:, 0:HALF] = SW[ROWS0, 0:HALF]
    wh[:, HALF:N] = SW[ROWS1, HALF:N]
    bfv = np.asarray(b, np.float32).reshape(N)
    bf = np.zeros((2 * NH, BPAD), np.float32)
    bf[0:HALF, 0] = bfv[0:HALF]
    bf[NH : NH + HALF, 0] = bfv[HALF:N]

    in_maps = []
    for i in range(N_CORES):
        xt = xf[i * SHARD : (i + 1) * SHARD].T          # [208, SHARD] view
        xh = np.empty((NR, SHARD), NP_BF16)
        xh[0:K] = xt[N - K : N]
        xh[K : N + K] = xt
        xh[N + K : NR] = xt[0:K]
        in_maps.append({"xh": xh, "wh": wh, "bias": bf})
    res = run_bass_kernel_spmd(nc, in_maps, core_ids=list(range(N_CORES)))
    LAST_RESULTS = res
    out = np.empty((ROWS_TOTAL, N), np.float32)
    for i, r in enumerate(res.results):
        yt = r["outt"]                                  # [224, SHARD] bf16
        out[i * SHARD : (i + 1) * SHARD, 0:HALF] = yt[0:HALF].T
        out[i * SHARD : (i + 1) * SHARD, HALF:N] = yt[NH : NH + HALF].T
    return out.reshape(B, T, N)


# revision 27
# speedup vs baseline: 1.0111x; 1.0111x over previous
"""Locally-connected graph-conv kernel for Trainium2 (Bass/Tile).

Computes out[b,t,m] = sum_n x[b,t,n] * (S*W)[n,m] + bias[m] for
x [64, 2048, 208], W/S [208, 208], bias [208].

The ring-graph support S is a +-4 band (mod 208), so each half of the
output nodes only needs a 112-row slice of the contraction dim. With a
rotated node layout (row j holds node (j-4) mod 208, 216 rows total):
  block 0 (m 0..103):   rotated rows   0..111
  block 1 (m 104..207): rotated rows 104..215
Each output block is a SINGLE [112,104] x [112,512] matmul with the
host-premasked weight block stationary in the PE array and x^T streaming
as the moving operand.

Everything that touches HBM is bf16 (PSUM accumulation stays fp32): the
2e-2 rel-err budget dwarfs bf16 rounding (~5e-3) and it halves DMA bytes
vs fp32. HBM per NeuronCore is ~358 GB/s and ramps up over the first
~20 us, so the ~14.9 MB/core of traffic floors the kernel at ~45 us.
The two output blocks form fully decoupled pipelines that share only
the PE and the load ring:
  block 0: Sync-ring load -> matmul -> VectorE evict -> Sync-ring store
  block 1: Sync-ring load -> matmul -> ScalarE evict -> Scalar-ring store
so neither evicting engine ever waits on the other. Block 0 stores are
queued on the Sync ring AFTER all loads (FIFO keeps them out of the
loads' way; the ring is otherwise idle then). Weights and bias are
padded to >=1KB DMA rows (tiny-descriptor DMAs crawl at ~27 GB/s) and
ride the Scalar ring ahead of the stores; GpSimd SWDGE is avoided (its
completion semaphore fires ~7 us late). PSUM->SBUF eviction is stuck at
1 elem/lane/cycle (fp32 PSUM source), which is why the eviction work is
split across both engines. A few dummy matmuls on the weight tile right
after it lands warm the PE HAM clock gate (cold 1.2 GHz -> warm 2.4).
The host transposes y^T back at gather.
"""

import numpy as np
import ml_dtypes
from contextlib import ExitStack

import concourse.bacc as bacc
import concourse.mybir as mybir
import concourse.tile as tile
from concourse.bass_utils import run_bass_kernel_spmd

N = 208                      # nodes
HALF = 104                   # output nodes per block
K = 4                        # band half-width of S
NH = 2 * K + HALF            # 112 contraction rows per block (halo incl.)
NR = N + 2 * K               # 216 rotated rows
WPAD = 1024                  # wh DRAM row padding (2 KB rows -> fast DMA)
BPAD = 256                   # bias DRAM row padding (1 KB f32 rows)
N_CORES = 8
B, T = 64, 2048
ROWS_TOTAL = B * T           # 131072
SHARD = ROWS_TOTAL // N_CORES    # 16384 rows per core
TB = 512                     # moving-block columns per matmul (fp32 PSUM max)
TB2 = 2 * TB                 # eviction group (2 PSUM banks)
CHUNKS = [2048, 2048, 4096, 4096, 2048, 2048]   # t-cols per pipeline chunk
assert sum(CHUNKS) == SHARD
N_DUMMY = 8                  # PE warm-up matmuls on the weight tile

FP32 = mybir.dt.float32
BF16 = mybir.dt.bfloat16
NP_BF16 = ml_dtypes.bfloat16
IDENT = mybir.ActivationFunctionType.Identity

# halo row order (indices into the [208] node dim) for each block
ROWS0 = list(range(N - K, N)) + list(range(0, HALF + K))          # 112
ROWS1 = list(range(HALF - K, N)) + list(range(0, K))              # 112

_CACHE = {}
LAST_RESULTS = None          # BassKernelResults of the most recent run


def _kernel_body(tc):
    nc = tc.nc
    # rotated x: row j = node (j-4) mod 208; block0 = rows 0:112,
    # block1 = rows 104:216
    x_d = nc.dram_tensor("xh", [NR, SHARD], BF16, kind="ExternalInput").ap()
    w_d = nc.dram_tensor("wh", [NH, WPAD], BF16, kind="ExternalInput").ap()
    b_d = nc.dram_tensor("bias", [2 * NH, BPAD], FP32, kind="ExternalInput").ap()
    o_d = nc.dram_tensor("outt", [2 * NH, SHARD], BF16, kind="ExternalOutput").ap()

    with ExitStack() as ctx:
        const = ctx.enter_context(tc.tile_pool(name="const", bufs=1))

        # One-time setup on the Scalar ring (it carries only stores
        # later): weights first (the warm-up matmuls need them), then
        # bias -- delays no x load, lands before the first eviction.
        wh = const.tile([NH, WPAD], BF16, tag="wh")
        nc.scalar.dma_start(wh, w_d)
        bA = const.tile([NH, BPAD], FP32, tag="bA")
        bB = const.tile([NH, BPAD], FP32, tag="bB")
        nc.scalar.dma_start(bA, b_d[0:NH, :])
        nc.scalar.dma_start(bB, b_d[NH : 2 * NH, :])
        bAc = bA[0:HALF, 0:1]
        bBc = bB[0:HALF, 0:1]

        o0p = ctx.enter_context(tc.tile_pool(name="o0p", bufs=3))
        o1p = ctx.enter_context(tc.tile_pool(name="o1p", bufs=3))
        ps0p = ctx.enter_context(tc.tile_pool(name="ps0p", bufs=2, space="PSUM"))
        ps1p = ctx.enter_context(tc.tile_pool(name="ps1p", bufs=2, space="PSUM"))

        # All x loads up-front on the Sync ring in consumption order
        # (block0 then block1 per chunk) into persistent tiles; the
        # first chunk is split across both rings for a faster start.
        xts = []
        col = 0
        for c, csz in enumerate(CHUNKS):
            lsl = slice(col, col + csz)
            xh0 = const.tile([NH, csz], BF16, tag=f"xh0_{c}")
            xh1 = const.tile([NH, csz], BF16, tag=f"xh1_{c}")
            if c == 0:
                nc.sync.dma_start(xh0[0:64, :], x_d[0:64, lsl])
                nc.scalar.dma_start(xh0[64:NH, :], x_d[64:NH, lsl])
                nc.sync.dma_start(xh1[0:64, :], x_d[HALF : HALF + 64, lsl])
                nc.scalar.dma_start(xh1[64:NH, :], x_d[HALF + 64 : NR, lsl])
            else:
                nc.sync.dma_start(xh0, x_d[0:NH, lsl])
                nc.sync.dma_start(xh1, x_d[HALF:NR, lsl])
            xts.append((xh0, xh1, col, csz))
            col += csz

        # PE warm-up: HAM un-throttles (1.2 -> 2.4 GHz) after ~3.4us of
        # sustained busy; burn idle pre-data time on the weight tile.
        for _ in range(N_DUMMY):
            psd = ps0p.tile([HALF, TB2], FP32, tag="ps0")
            nc.tensor.matmul(psd[:, 0:TB], wh[:, 0:HALF], wh[:, 0:TB], start=True, stop=True)

        for c, (xh0, xh1, col, csz) in enumerate(xts):
            tsl = slice(col, col + csz)
            o0_t = o0p.tile([NH, max(CHUNKS)], BF16, tag="o0")
            o1_t = o1p.tile([NH, max(CHUNKS)], BF16, tag="o1")
            for s in range(csz // TB2):
                g = slice(s * TB2, (s + 1) * TB2)
                ga = slice(s * TB2, s * TB2 + TB)
                gb = slice(s * TB2 + TB, (s + 1) * TB2)
                # [104, 1024] PSUM tiles (2 banks); one matmul per bank
                ps0 = ps0p.tile([HALF, TB2], FP32, tag="ps0")
                nc.tensor.matmul(ps0[:, 0:TB], wh[:, 0:HALF], xh0[:, ga], start=True, stop=True)
                nc.tensor.matmul(ps0[:, TB:TB2], wh[:, 0:HALF], xh0[:, gb], start=True, stop=True)
                ps1 = ps1p.tile([HALF, TB2], FP32, tag="ps1")
                nc.tensor.matmul(ps1[:, 0:TB], wh[:, HALF:N], xh1[:, ga], start=True, stop=True)
                nc.tensor.matmul(ps1[:, TB:TB2], wh[:, HALF:N], xh1[:, gb], start=True, stop=True)
                # evictions split across engines; both fuse bias + fp32->bf16
                nc.vector.tensor_scalar_add(o0_t[0:HALF, g], ps0, bAc)
                nc.scalar.activation(o1_t[0:HALF, g], ps1, IDENT, bias=bBc)
            # decoupled stores: block0 on the Sync ring (queued after all
            # loads), block1 on the Scalar ring. The last two block0
            # stores ride Scalar too -- by then its b1 stores are done
            # while the Sync ring is still draining earlier b0 stores.
            eng0 = nc.scalar if c >= len(CHUNKS) - 2 else nc.sync
            eng0.dma_start(o_d[0:NH, tsl], o0_t[:, 0:csz])
            nc.scalar.dma_start(o_d[NH : 2 * NH, tsl], o1_t[:, 0:csz])


def _build():
    nc = bacc.Bacc(
        "TRN2",
        target_bir_lowering=False,
        debug=False,
        num_devices=N_CORES,
    )
    with tile.TileContext(nc) as tc:
        _kernel_body(tc)
    nc.compile()
    return nc


def kernel(x, W, b, S):
    global LAST_RESULTS
    nc = _CACHE.get("nc")
    if nc is None:
        nc = _build()
        _CACHE["nc"] = nc

    xf = np.asarray(x, np.float32).reshape(ROWS_TOTAL, N)
    SW = (np.asarray(S, np.float32) * np.asarray(W, np.float32))
    wh = np.zeros((NH, WPAD), NP_BF16)
    wh[:, 0:HALF] = SW[ROWS0, 0:HALF]
    wh[:, HALF:N] = SW[ROWS1, HALF:N]
    bfv = np.asarray(b, np.float32).reshape(N)
    bf = np.zeros((2 * NH, BPAD), np.float32)
    bf[0:HALF, 0] = bfv[0:HALF]
    bf[NH : NH + HALF, 0] = bfv[HALF:N]

    in_maps = []
    for i in range(N_CORES):
        xt = xf[i * SHARD : (i + 1) * SHARD].T          # [208, SHARD] view
        xh = np.empty((NR, SHARD), NP_BF16)
        xh[0:K] = xt[N - K : N]
        xh[K : N + K] = xt
        xh[N + K : NR] = xt[0:K]
        in_maps.append({"xh": xh, "wh": wh, "bias": bf})
    res = run_bass_kernel_spmd(nc, in_maps, core_ids=list(range(N_CORES)))
    LAST_RESULTS = res
    out = np.empty((ROWS_TOTAL, N), np.float32)
    for i, r in enumerate(res.results):
        yt = r["outt"]                                  # [224, SHARD] bf16
        out[i * SHARD : (i + 1) * SHARD, 0:HALF] = yt[0:HALF].T
        out[i * SHARD : (i + 1) * SHARD, HALF:N] = yt[NH : NH + HALF].T
    return out.reshape(B, T, N)


# revision 28
# speedup vs baseline: 1.1030x; 1.0908x over previous
"""Locally-connected graph-conv kernel for Trainium2 (Bass/Tile).

Computes out[b,t,m] = sum_n x[b,t,n] * (S*W)[n,m] + bias[m] for
x [64, 2048, 208], W/S [208, 208], bias [208].

The ring-graph support S is a +-4 band (mod 208), so each half of the
output nodes only needs a 112-row slice of the contraction dim. With a
rotated node layout (row j holds node (j-4) mod 208, 216 rows total):
  block 0 (m 0..103):   rotated rows   0..111
  block 1 (m 104..207): rotated rows 104..215
Each output block is a SINGLE [112,104] x [112,512] matmul with the
host-premasked weight block stationary in the PE array and x^T streaming
as the moving operand.

Everything that touches HBM is bf16 (PSUM accumulation stays fp32):
the 2e-2 rel-err budget dwarfs bf16 rounding (~5e-3), and it halves DMA
bytes vs fp32. HBM per NeuronCore is ~358 GB/s (and ramps up over the
first ~20 us), so the ~14.9 MB/core of traffic floors the kernel at
~45 us; everything else is shaped to stay under that:
 - all x loads are issued up-front into persistent SBUF tiles (x fits:
   2 x 32 KB/partition) on the Sync ring, so the load stream runs at
   whatever rate HBM gives with zero dependency stalls; graduated chunk
   sizes (2 KB cols first, 4 KB mid) start compute early and keep the
   pipeline tail short.
 - weights/bias DRAM rows are padded to >=1 KB so their one-time loads
   are not tiny-descriptor crawls that clog a ring (wh first on Scalar,
   bias on GpSimd).
 - PSUM->SBUF eviction is stuck at 1 elem/lane/cycle (fp32 PSUM source),
   so block 0 evicts on VectorE and block 1 on ScalarE. The two blocks
   form decoupled pipelines sharing only the PE and the load ring:
   block 0 stores ride the Sync ring (queued after all loads), block 1
   stores the Scalar ring, so neither evicting engine ever blocks on
   the other's semaphore.
 - 4 dummy matmuls on the weight tile right after it lands warm the PE
   HAM (cold 1.2 GHz -> warm 2.4 GHz) before real data arrives.
The host transposes y^T back at gather.
"""

import numpy as np
import ml_dtypes
from contextlib import ExitStack

import concourse.bacc as bacc
import concourse.mybir as mybir
import concourse.tile as tile
from concourse.bass_utils import run_bass_kernel_spmd

N = 208                      # nodes
HALF = 104                   # output nodes per block
K = 4                        # band half-width of S
NH = 2 * K + HALF            # 112 contraction rows per block (halo incl.)
NR = N + 2 * K               # 216 rotated rows
WPAD = 1024                  # wh DRAM row padding (2 KB rows -> fast DMA)
BPAD = 512                   # bias DRAM row padding (2 KB f32 rows)
N_CORES = 8
B, T = 64, 2048
ROWS_TOTAL = B * T           # 131072
SHARD = ROWS_TOTAL // N_CORES    # 16384 rows per core
TB = 512                     # moving-block columns per matmul (fp32 PSUM max)
TB2 = 2 * TB                 # eviction group (2 PSUM banks)
CHUNKS = [2048, 2048, 4096, 4096, 2048, 2048]   # t-cols per pipeline chunk
assert sum(CHUNKS) == SHARD
N_DUMMY = 4                  # PE warm-up matmuls on the weight tile

FP32 = mybir.dt.float32
BF16 = mybir.dt.bfloat16
NP_BF16 = ml_dtypes.bfloat16
IDENT = mybir.ActivationFunctionType.Identity

# halo row order (indices into the [208] node dim) for each block
ROWS0 = list(range(N - K, N)) + list(range(0, HALF + K))          # 112
ROWS1 = list(range(HALF - K, N)) + list(range(0, K))              # 112

_CACHE = {}
LAST_RESULTS = None          # BassKernelResults of the most recent run


def _kernel_body(tc):
    nc = tc.nc
    # rotated x: row j = node (j-4) mod 208; block0 = rows 0:112,
    # block1 = rows 104:216
    x_d = nc.dram_tensor("xh", [NR, SHARD], BF16, kind="ExternalInput").ap()
    w_d = nc.dram_tensor("wh", [NH, WPAD], BF16, kind="ExternalInput").ap()
    b_d = nc.dram_tensor("bias", [2 * NH, BPAD], FP32, kind="ExternalInput").ap()
    o_d = nc.dram_tensor("outt", [2 * NH, SHARD], BF16, kind="ExternalOutput").ap()

    with ExitStack() as ctx:
        const = ctx.enter_context(tc.tile_pool(name="const", bufs=1))

        # One-time setup: host-premasked halo-ordered weights (2 KB rows,
        # one fast DMA) first on the Scalar ring; bias halves on GpSimd
        # (off every latency-critical path).
        wh = const.tile([NH, WPAD], BF16, tag="wh")
        nc.scalar.dma_start(wh, w_d)
        bA = const.tile([NH, BPAD], FP32, tag="bA")
        bB = const.tile([NH, BPAD], FP32, tag="bB")
        nc.gpsimd.dma_start(bA, b_d[0:NH, :])
        nc.gpsimd.dma_start(bB, b_d[NH : 2 * NH, :])
        bAc = bA[0:HALF, 0:1]
        bBc = bB[0:HALF, 0:1]

        o0p = ctx.enter_context(tc.tile_pool(name="o0p", bufs=3))
        o1p = ctx.enter_context(tc.tile_pool(name="o1p", bufs=3))
        ps0p = ctx.enter_context(tc.tile_pool(name="ps0p", bufs=2, space="PSUM"))
        ps1p = ctx.enter_context(tc.tile_pool(name="ps1p", bufs=2, space="PSUM"))

        # All x loads up-front on the Sync ring into persistent tiles
        # (first chunk split across both rings for a faster start).
        xts = []
        col = 0
        for c, csz in enumerate(CHUNKS):
            lsl = slice(col, col + csz)
            xh0 = const.tile([NH, csz], BF16, tag=f"xh0_{c}")
            xh1 = const.tile([NH, csz], BF16, tag=f"xh1_{c}")
            if c == 0:
                nc.sync.dma_start(xh0[0:64, :], x_d[0:64, lsl])
                nc.scalar.dma_start(xh0[64:NH, :], x_d[64:NH, lsl])
                nc.sync.dma_start(xh1[0:64, :], x_d[HALF : HALF + 64, lsl])
                nc.scalar.dma_start(xh1[64:NH, :], x_d[HALF + 64 : NR, lsl])
            else:
                nc.sync.dma_start(xh0, x_d[0:NH, lsl])
                nc.sync.dma_start(xh1, x_d[HALF:NR, lsl])
            xts.append((xh0, xh1, col, csz))
            col += csz

        # PE warm-up: HAM un-throttles (1.2 -> 2.4 GHz) after ~3.4us of
        # sustained busy; burn idle pre-data time on the weight tile.
        for _ in range(N_DUMMY):
            psd = ps0p.tile([HALF, TB2], FP32, tag="ps0")
            nc.tensor.matmul(psd[:, 0:TB], wh[:, 0:HALF], wh[:, 0:TB], start=True, stop=True)

        for c, (xh0, xh1, col, csz) in enumerate(xts):
            tsl = slice(col, col + csz)
            o0_t = o0p.tile([NH, max(CHUNKS)], BF16, tag="o0")
            o1_t = o1p.tile([NH, max(CHUNKS)], BF16, tag="o1")
            for s in range(csz // TB2):
                g = slice(s * TB2, (s + 1) * TB2)
                ga = slice(s * TB2, s * TB2 + TB)
                gb = slice(s * TB2 + TB, (s + 1) * TB2)
                # [104, 1024] PSUM tiles (2 banks); each matmul fills one bank
                ps0 = ps0p.tile([HALF, TB2], FP32, tag="ps0")
                nc.tensor.matmul(ps0[:, 0:TB], wh[:, 0:HALF], xh0[:, ga], start=True, stop=True)
                nc.tensor.matmul(ps0[:, TB:TB2], wh[:, 0:HALF], xh0[:, gb], start=True, stop=True)
                ps1 = ps1p.tile([HALF, TB2], FP32, tag="ps1")
                nc.tensor.matmul(ps1[:, 0:TB], wh[:, HALF:N], xh1[:, ga], start=True, stop=True)
                nc.tensor.matmul(ps1[:, TB:TB2], wh[:, HALF:N], xh1[:, gb], start=True, stop=True)
                # evictions split across engines: block0 on VectorE,
                # block1 on ScalarE; both fuse the bias and fp32->bf16
                nc.vector.tensor_scalar_add(o0_t[0:HALF, g], ps0, bAc)
                nc.scalar.activation(o1_t[0:HALF, g], ps1, IDENT, bias=bBc)
            # non-overlapping stores on separate rings: block0 on Sync
            # (rides behind the loads), block1 on Scalar
            nc.sync.dma_start(o_d[0:NH, tsl], o0_t[:, 0:csz])
            nc.scalar.dma_start(o_d[NH : 2 * NH, tsl], o1_t[:, 0:csz])


def _build():
    nc = bacc.Bacc(
        "TRN2",
        target_bir_lowering=False,
        debug=False,
        num_devices=N_CORES,
    )
    with tile.TileContext(nc) as tc:
        _kernel_body(tc)
    nc.compile()
    return nc


def kernel(x, W, b, S):
    global LAST_RESULTS
    nc = _CACHE.get("nc")
    if nc is None:
        nc = _build()
        _CACHE["nc"] = nc

    xf = np.asarray(x, np.float32).reshape(ROWS_TOTAL, N)
    SW = (np.asarray(S, np.float32) * np.asarray(W, np.float32))
    wh = np.zeros((NH, WPAD), NP_BF16)
    wh[:, 0:HALF] = SW[ROWS0, 0:HALF]
    wh[:, HALF:N] = SW[ROWS1, HALF:N]
    bfv = np.asarray(b, np.float32).reshape(N)
    bf = np.zeros((2 * NH, BPAD), np.float32)
    bf[0:HALF, 0] = bfv[0:HALF]
    bf[NH : NH + HALF, 0] = bfv[HALF:N]

    in_maps = []
    for i in range(N_CORES):
        xt = xf[i * SHARD : (i + 1) * SHARD].T          # [208, SHARD] view
        xh = np.empty((NR, SHARD), NP_BF16)
        xh[0:K] = xt[N - K : N]
        xh[K : N + K] = xt
        xh[N + K : NR] = xt[0:K]
        in_maps.append({"xh": xh, "wh": wh, "bias": bf})
    res = run_bass_kernel_spmd(nc, in_maps, core_ids=list(range(N_CORES)))
    LAST_RESULTS = res
    out = np.empty((ROWS_TOTAL, N), np.float32)
    for i, r in enumerate(res.results):
        yt = r["outt"]                                  # [224, SHARD] bf16
        out[i * SHARD : (i + 1) * SHARD, 0:HALF] = yt[0:HALF].T
        out[i * SHARD : (i + 1) * SHARD, HALF:N] = yt[NH : NH + HALF].T
    return out.reshape(B, T, N)
